# revision 1
# baseline (speedup 1.0000x reference)
"""Trainium2 Bass kernel for nn_DCTFeatureModel.

Math: the reference pipeline (3D DCT-II over [time-in-bin, H, W], mean over
DCT bins, full-receptive-field Conv3d, bias, LeakyReLU) is linear up to the
LeakyReLU, so everything folds into a single small matmul:

    feat[b,s,o] = LeakyReLU( sum_{c,t,i,j} x[b,s,c,t,i,j] * Weff[s,o,t,i,j]
                             + bias[s,o] )
    Weff[s,o,t,i,j] = (1/8) * sum_{f,p,q} Ct[f,t] Cs[p,i] Cs[q,j] W[s,o,f,p,q]

Weff is tiny (2*64*2048 floats) and computed on host. The device kernel is
memory-bound: stream x (134 MB full / 16.8 MB per core), reduce over the 8
DCT bins (c), then a [128b x 2048k] @ [2048k x 64o] matmul per subwindow.

Sharding: pure data-parallel over batch, 1024/8 = 128 rows per core.
"""

import os
from contextlib import ExitStack

import numpy as np

import concourse.bacc as bacc
import concourse.tile as tile
from concourse import masks, mybir
from concourse.bass_utils import run_bass_kernel_spmd

# Problem shapes (hardcoded per contract)
B = 1024
NCORES = 8
BS = B // NCORES          # 128 batch rows per core
NSW = 2                   # subwindows
NBINS = 8                 # DCT bins (mean-reduced)
NDCT = 32                 # time points per bin
HW = 8
NF = 64                   # conv output filters per subwindow
K = NDCT * HW * HW        # 2048 contraction elements per (s, c)
P = 128                   # partitions
NCHUNK = K // P           # 16 k-chunks of 128
OUT_F = NSW * NF          # 128 output features
SLOPE = 0.02

F32 = mybir.dt.float32

_cached = None            # (nc,) built once per process
last_results = None       # BassKernelResults of the most recent run (for test harness)


def _dct2(N):
    n = np.arange(N, dtype=np.float64)
    k = np.arange(N, dtype=np.float64)
    return 2.0 * np.cos(np.pi * (2.0 * n[None, :] + 1.0) * k[:, None] / (2.0 * N))


def _kernel_body(tc, x, w, bias, out):
    """Per-core tile kernel.

    x:    DRAM [BS, NSW*NBINS*K]  (b, (s c k)) f32
    w:    DRAM [P, NSW*NCHUNK*NF] Weff chunked: w[p, (s ch o)] = Weff[s, ch*128+p, o]
    bias: DRAM [1, OUT_F]
    out:  DRAM [BS, OUT_F]
    """
    nc = tc.nc
    with ExitStack() as ctx:
        const_pool = ctx.enter_context(tc.tile_pool(name="const", bufs=1))
        xpool = ctx.enter_context(tc.tile_pool(name="xp", bufs=6))
        zpool = ctx.enter_context(tc.tile_pool(name="zp", bufs=2))
        ztpool = ctx.enter_context(tc.tile_pool(name="ztp", bufs=3))
        opool = ctx.enter_context(tc.tile_pool(name="op", bufs=1))
        ptr_pool = ctx.enter_context(tc.tile_pool(name="ptr", bufs=2, space="PSUM"))
        pft_pool = ctx.enter_context(tc.tile_pool(name="pft", bufs=2, space="PSUM"))

        ident = const_pool.tile([P, P], F32)
        masks.make_identity(nc, ident)
        w_sb = const_pool.tile_from(w)        # [128, NSW*NCHUNK*NF]
        bias_sb = const_pool.tile_from(bias)  # [1, 128]
        ones = const_pool.tile([1, P], F32)
        nc.vector.memset(ones, 1.0)

        out_sb = opool.tile([BS, OUT_F], F32)

        for s in range(NSW):
            # --- c-reduction: z[b, k] = sum_c x[b, s, c, k] ---
            z = zpool.tile([P, K], F32, tag="z")
            t0 = xpool.tile([P, K], F32, tag="x")
            nc.sync.dma_start(out=t0, in_=x[:, (s * NBINS + 0) * K:(s * NBINS + 1) * K])
            t1 = xpool.tile([P, K], F32, tag="x")
            nc.sync.dma_start(out=t1, in_=x[:, (s * NBINS + 1) * K:(s * NBINS + 2) * K])
            nc.vector.tensor_add(out=z, in0=t0, in1=t1)
            for c in range(2, NBINS):
                t = xpool.tile([P, K], F32, tag="x")
                nc.sync.dma_start(
                    out=t, in_=x[:, (s * NBINS + c) * K:(s * NBINS + c + 1) * K]
                )
                nc.vector.tensor_add(out=z, in0=z, in1=t)

            # --- feat[s] = z @ Weff[s] via PE: transpose k-chunks, accumulate ---
            psum_feat = pft_pool.tile([P, NF], F32, tag="feat")
            for ch in range(NCHUNK):
                pt = ptr_pool.tile([P, P], F32, tag="tr")
                nc.tensor.transpose(pt, z[:, ch * P:(ch + 1) * P], ident)
                zt = ztpool.tile([P, P], F32, tag="zt")
                nc.scalar.copy(zt, pt)
                nc.tensor.matmul(
                    psum_feat,
                    lhsT=zt,
                    rhs=w_sb[:, (s * NCHUNK + ch) * NF:(s * NCHUNK + ch + 1) * NF],
                    start=(ch == 0),
                    stop=False,
                )
            # bias via rank-1 matmul: ones[1, b].T @ bias[1, o]
            nc.tensor.matmul(
                psum_feat,
                lhsT=ones,
                rhs=bias_sb[:, s * NF:(s + 1) * NF],
                start=False,
                stop=True,
            )
            # LeakyReLU(v) = max(v, slope*v)  (slope < 1)
            tmp = ztpool.tile([P, NF], F32, tag="lrelu")
            nc.vector.tensor_scalar_mul(tmp, psum_feat, SLOPE)
            nc.vector.tensor_max(
                out=out_sb[:, s * NF:(s + 1) * NF], in0=psum_feat, in1=tmp
            )

        nc.sync.dma_start(out=out, in_=out_sb)


def _build():
    global _cached
    if _cached is not None:
        return _cached
    nc = bacc.Bacc(
        "TRN2",
        target_bir_lowering=False,
        debug=False,
        enable_asserts=False,
        num_devices=NCORES,
    )
    x_ap = nc.dram_tensor("x", [BS, NSW * NBINS * K], F32, kind="ExternalInput").ap()
    w_ap = nc.dram_tensor("w", [P, NSW * NCHUNK * NF], F32, kind="ExternalInput").ap()
    b_ap = nc.dram_tensor("bias", [1, OUT_F], F32, kind="ExternalInput").ap()
    out_ap = nc.dram_tensor("out", [BS, OUT_F], F32, kind="ExternalOutput").ap()
    with tile.TileContext(nc, trace_sim=False) as tc:
        _kernel_body(tc, x_ap, w_ap, b_ap, out_ap)
    nc.compile()
    _cached = nc
    return nc


def kernel(x, W, b):
    global last_results
    assert x.shape == (B, 1, NSW * NBINS * NDCT, HW, HW), x.shape
    nc = _build()

    # Host-side folding of the DCT matrices into the conv weights (tiny).
    Ct = _dct2(NDCT)                       # [f, t]
    Cs = _dct2(HW)                         # [p, i]
    Weff = np.einsum(
        "ft,pi,qj,sofpq->sotij", Ct, Cs, Cs, W.astype(np.float64), optimize=True
    ) / float(NBINS)
    Weff_k = Weff.reshape(NSW, NF, K)      # [s, o, k]
    # device layout: w[p, s*NCHUNK*NF + ch*NF + o] = Weff_k[s, o, ch*128 + p]
    w_dev = np.ascontiguousarray(
        Weff_k.reshape(NSW, NF, NCHUNK, P).transpose(3, 0, 2, 1).reshape(P, NSW * NCHUNK * NF)
    ).astype(np.float32)
    bias_dev = np.ascontiguousarray(b.reshape(1, OUT_F)).astype(np.float32)

    x2 = np.ascontiguousarray(x.reshape(B, NSW * NBINS * K)).astype(np.float32)
    in_maps = [
        {"x": x2[i * BS:(i + 1) * BS], "w": w_dev, "bias": bias_dev}
        for i in range(NCORES)
    ]
    res = run_bass_kernel_spmd(nc, in_maps, core_ids=list(range(NCORES)))
    last_results = res
    return np.concatenate([r["out"] for r in res.results], axis=0)
